# revision 16
# baseline (speedup 1.0000x reference)
"""Trainium2 8-core kernel for nn_CellInteract.

out = ((exp(-sqr_pdist/L^2) * sigmoid(enc @ T @ enc.T)) @ expr) @ G / d_gene

Strategy:
  - Rewrite as gated @ E' with E' = expr @ G / d_gene (associativity), so the
    gated matrix feeds exactly one matmul and no transpose of the NxD partial
    product is ever needed.
  - Shard rows (cells) across 8 cores. Each core computes E' for its own row
    block (1/8 of the flops); 8 chunked AllGathers (one per 128-row tile of
    the local E') replicate it while the next tile is still being computed.
  - The main loop walks j-chunks grouped by AllGather chunk (jc = t*8 + c):
    all work gated on AllGather c happens in "c-phase" c, so compute starts
    as soon as the first chunk lands and the remaining collectives stream in
    behind the matmuls.
  - Scores are computed in transposed layout ST[j, i_local] = enc @ A.T with
    A = enc_local @ T, via float32r matmuls (TF32-like precision at full PE
    rate). That puts the contraction index j on partitions, which is the
    layout the O-matmul needs for its stationary operand.
  - Spatial gate: d in [0,1), so exp(-d/1e4) == 1 - d*1e-4 to ~5e-9; computed
    on VectorE as a fused multiply-add, keeping ScalarE free for the sigmoid.
  - O accumulates in PSUM within a c-phase and drains to an SBUF accumulator,
    freeing the PSUM banks so successive c-phases (and i-blocks) pipeline.
  - DMAs are packed 4-8 iterations per issue to keep the HWDGE sequencer off
    the critical path.
"""

import sys

for _p in ("/opt/trn_rl_repo", "/root/.axon_site"):
    if _p not in sys.path:
        sys.path.insert(0, _p)

import numpy as np
import ml_dtypes

import concourse.bacc as bacc
import concourse.mybir as mybir
import concourse.tile as tile
from concourse.bass_utils import run_bass_kernel_spmd

N = 8192
D_GENE = 1024
D_EMBED = 256
N_CORES = 8
N_LOC = N // N_CORES          # 1024 rows per core
IB = 256                      # i-block
N_IB = N_LOC // IB            # 4
JC = 128                      # j-chunk (partition dim of ST tiles)
N_JC = N // JC                # 64
NC8 = 8                       # AllGather chunks == cores
NT = N_JC // NC8              # 8 t-iterations per c-phase
TG = 4                        # t per packed DMA group
INV_SPATIAL = -1e-4           # -1/LENGTH_SCALE^2
F32 = mybir.dt.float32
F32R = mybir.dt.float32r
BF16 = mybir.dt.bfloat16

_cached = {}


def _phase_a(nc, pa, ecp, ps, rp, dp, enclT, tfm, exprT, g):
    """AT = (enc_local @ T).T in f32r; E'_local = expr_local @ G / d in bf16,
    replicated via 8 chunked AllGathers pipelined with the compute.
    Returns (at_tiles, cc_out_list)."""
    AF = mybir.ActivationFunctionType
    ALU = mybir.AluOpType

    # ---- AT[e,i] = sum_d T[d,e] * enclT[d,i]; K=D_EMBED in 2 chunks ----
    tfm_t = [pa.tile([128, D_EMBED], F32R, tag=f"tfm{k}", name=f"tfm{k}")
             for k in range(2)]
    enclT_t = [pa.tile([128, N_LOC], F32R, tag=f"enclT{k}", name=f"enclT{k}")
               for k in range(2)]
    for k in range(2):
        nc.sync.dma_start(tfm_t[k][:], tfm[k * 128:(k + 1) * 128, :])
        nc.sync.dma_start(enclT_t[k][:], enclT[k * 128:(k + 1) * 128, :])
    at = [rp.tile([128, N_LOC], F32R, tag=f"at{e}", name=f"at{e}")
          for e in range(2)]
    for e in range(2):                 # output e-chunk (partition dim)
        for ih in range(2):            # N_LOC in halves of 512
            mm = ps.tile([128, 512], F32, tag="st", name="mm")
            for k in range(2):
                nc.tensor.matmul(
                    mm[:],
                    tfm_t[k][:, e * 128:(e + 1) * 128],
                    enclT_t[k][:, ih * 512:(ih + 1) * 512],
                    start=(k == 0), stop=(k == 1),
                )
            nc.scalar.activation(
                at[e][:, ih * 512:(ih + 1) * 512], mm[:], AF.Copy)

    # ---- E'_local = expr_local @ G / d_gene, AllGathered chunk by chunk ----
    g_t = [pa.tile([128, D_GENE], BF16, tag=f"g{k}", name=f"g{k}")
           for k in range(8)]
    for k in range(8):
        nc.sync.dma_start(g_t[k][:], g[k * 128:(k + 1) * 128, :])
    exprT_r = exprT.rearrange("(k p) j -> p k j", p=128)   # [128, 8, 1024]
    cc_out = []
    for jt in range(8):
        xtp = ecp.tile([128, 8 * 128], BF16, tag="xtp", name="xtp")
        nc.sync.dma_start(
            xtp[:], exprT_r[:, :, jt * 128:(jt + 1) * 128])
        ec = ecp.tile([128, D_GENE], BF16, tag="ec", name="ec")
        for gh in range(2):
            mm = ps.tile([128, 512], F32, tag="st", name="mm")
            for k in range(8):
                nc.tensor.matmul(
                    mm[:],
                    xtp[:, k * 128:(k + 1) * 128],
                    g_t[k][:, gh * 512:(gh + 1) * 512],
                    start=(k == 0), stop=(k == 7),
                )
            nc.scalar.activation(
                ec[:, gh * 512:(gh + 1) * 512], mm[:], AF.Copy,
                scale=1.0 / D_GENE)
        cc_in_jt = dp.tile([128, D_GENE], BF16, name=f"cc_in{jt}")
        cc_out_jt = dp.tile([N_CORES * 128, D_GENE], BF16, name=f"cc_out{jt}",
                            addr_space="Shared")
        nc.scalar.dma_start(cc_in_jt[:], ec[:])
        nc.gpsimd.collective_compute(
            "AllGather",
            ALU.bypass,
            ins=[cc_in_jt.opt()],
            outs=[cc_out_jt.opt()],
            replica_groups=[list(range(N_CORES))],
        )
        cc_out.append(cc_out_jt)
    return at, cc_out


def build():
    nc = bacc.Bacc("TRN2", target_bir_lowering=False, debug=False,
                   num_devices=N_CORES)

    # encTp[p, k, j] = encoding.T[k*128+p, j]  (k-chunk packed for 1-DMA loads)
    encTp = nc.dram_tensor("encTp", [128, 2, N], F32R, kind="ExternalInput").ap()
    enclT = nc.dram_tensor("enclT", [D_EMBED, N_LOC], F32R, kind="ExternalInput").ap()
    tfm = nc.dram_tensor("tfm", [D_EMBED, D_EMBED], F32R, kind="ExternalInput").ap()
    pdT = nc.dram_tensor("pdT", [N, N_LOC], BF16, kind="ExternalInput").ap()
    exprT = nc.dram_tensor("exprT", [D_GENE, N_LOC], BF16, kind="ExternalInput").ap()
    g = nc.dram_tensor("g", [D_GENE, D_GENE], BF16, kind="ExternalInput").ap()
    out = nc.dram_tensor("out", [N_LOC, D_GENE], F32, kind="ExternalOutput").ap()

    AF = mybir.ActivationFunctionType
    ALU = mybir.AluOpType

    with tile.TileContext(nc) as tc:
        with (
            tc.tile_pool(name="res", bufs=1) as rp,
            tc.tile_pool(name="dram", bufs=1, space="DRAM") as dp,
            tc.tile_pool(name="ps", bufs=4, space="PSUM") as ps,
            tc.tile_pool(name="ops", bufs=1, space="PSUM") as ops,
        ):
            with (
                tc.tile_pool(name="pha", bufs=1) as pa,
                tc.tile_pool(name="ecp", bufs=2) as ecp,
            ):
                at, cc_out = _phase_a(nc, pa, ecp, ps, rp, dp,
                                      enclT, tfm, exprT, g)

            # O accumulators in SBUF (f32), one per i-block
            osb = [rp.tile([128, D_GENE], F32, tag=f"osb{si}",
                           name=f"osb{si}") for si in range(2 * N_IB)]

            # pdT rows j = t*1024 + c*128 + p  ->  [p, t, c, i]
            pdT_r = pdT.rearrange("(t c p) i -> p t c i", p=128, c=NC8)
            # encTp cols j = t*1024 + c*128 + jj  ->  [p, k, t, c, jj]
            encT_r = encTp.rearrange("p k (t c jj) -> p k t c jj",
                                     t=NT, c=NC8)

            # ---------------- main loop ----------------
            # i handled in pairs of blocks (IBP=512) so score matmuls run at
            # N=512; O accumulation split by g-half so PSUM stays at 8 banks
            # (4 x o + 4 x st). Gating tiles are computed once per (c, ibp)
            # and reused by both g-half passes. The scores/gating for the
            # first PRE c-phases are emitted before any O-matmul, giving the
            # PE useful work while the first AllGathers are still in flight.
            PRE = 2
            with tc.tile_pool(name="str", bufs=1) as ms:
                def gating(c, ibp):
                    """score + sigmoid + spatial gate for one (c, ibp);
                    returns the 8 gated lhsT tiles."""
                    i0 = ibp * 512
                    ekc = ms.tile([128, 2 * NT * JC], F32R,
                                  tag="ekc", name=f"ekc{c}", bufs=2)
                    nc.sync.dma_start(ekc[:], encT_r[:, :, :, c, :])
                    gts = []
                    for tg in range(2):
                        t0 = tg * TG
                        pdp = ms.tile([128, TG * 512], BF16,
                                      tag="pdp", name="pdp", bufs=2)
                        nc.sync.dma_start(
                            pdp[:], pdT_r[:, t0:t0 + TG, c, i0:i0 + 512])
                        spatp = ms.tile([128, TG * 512], BF16,
                                        tag="spatp", name="spatp", bufs=2)
                        nc.vector.tensor_scalar(
                            spatp[:], pdp[:], INV_SPATIAL, 1.0,
                            ALU.mult, ALU.add)
                        for dt in range(TG):
                            t = t0 + dt
                            st = ps.tile([JC, 512], F32, tag="st", name="st")
                            for k in range(2):
                                nc.tensor.matmul(
                                    st[:],
                                    ekc[:, (k * NT + t) * JC:
                                        (k * NT + t + 1) * JC],
                                    at[k][:, i0:i0 + 512],
                                    start=(k == 0), stop=(k == 1))
                            sig = ms.tile([JC, 512], BF16,
                                          tag="sig", name="sig", bufs=8)
                            nc.scalar.activation(sig[:], st[:], AF.Sigmoid)
                            gt = ms.tile([JC, 512], BF16,
                                         tag="gt", name="gt", bufs=70)
                            nc.vector.tensor_mul(
                                gt[:], sig[:], spatp[:, dt * 512:(dt + 1) * 512])
                            gts.append(gt)
                    return gts

                def load_ep(c):
                    # E' slice for AG chunk c: epc[:, t*1024:] = rank t's rows
                    # of chunked AllGather c (= E'[t*1024 + c*128 ..]). Loaded
                    # via GpSimd (SWDGE) so the collective-completion wait
                    # does not head-of-line-block the sync HWDGE queue.
                    epc = ms.tile([128, NT * D_GENE], BF16,
                                  tag="ep", name=f"ep{c}", bufs=2)
                    for t in range(NT):
                        nc.gpsimd.dma_start(
                            epc[:, t * D_GENE:(t + 1) * D_GENE],
                            cc_out[c][t * 128:(t + 1) * 128, :])
                    return epc

                def o_phase(c, ibp, epc, gts):
                    for gh in range(2):
                        o_ps = [ops.tile([128, 512], F32, tag=f"o{si}",
                                         name=f"o{si}") for si in range(4)]
                        for t in range(NT):
                            for si in range(4):
                                nc.tensor.matmul(
                                    o_ps[si][:],
                                    gts[t][:, si * 128:(si + 1) * 128],
                                    epc[:, t * D_GENE + gh * 512:
                                         t * D_GENE + (gh + 1) * 512],
                                    start=(t == 0), stop=(t == NT - 1),
                                )
                        for si in range(4):
                            dst = osb[4 * ibp + si][:, gh * 512:(gh + 1) * 512]
                            if c == 0:
                                nc.vector.tensor_copy(dst, o_ps[si][:])
                            else:
                                nc.vector.tensor_add(dst, dst, o_ps[si][:])

                # burst: scores/gating for the first PRE c-phases up front
                pre_gts = {}
                for c in range(PRE):
                    for ibp in range(2):
                        pre_gts[(c, ibp)] = gating(c, ibp)
                for c in range(PRE):
                    epc = load_ep(c)
                    for ibp in range(2):
                        o_phase(c, ibp, epc, pre_gts.pop((c, ibp)))
                for c in range(PRE, NC8):
                    epc = load_ep(c)
                    for ibp in range(2):
                        o_phase(c, ibp, epc, gating(c, ibp))

                # write out
                for si in range(2 * N_IB):
                    nc.sync.dma_start(
                        out[si * 128:(si + 1) * 128, :], osb[si][:])

    nc.compile()
    return nc


def _prep_inputs(expression, encoding, sqr_pdist, transform, gene_response):
    expression = np.asarray(expression, dtype=np.float32)
    encoding = np.asarray(encoding, dtype=np.float32)
    sqr_pdist = np.asarray(sqr_pdist, dtype=np.float32)
    transform = np.asarray(transform, dtype=np.float32)
    gene_response = np.asarray(gene_response, dtype=np.float32)

    encT = np.ascontiguousarray(encoding.T)                    # [256, 8192]
    encTp = np.ascontiguousarray(encT.reshape(2, 128, N).transpose(1, 0, 2))
    tfm = np.ascontiguousarray(transform)                      # [256, 256]
    g_bf = np.ascontiguousarray(gene_response.astype(ml_dtypes.bfloat16))
    in_maps = []
    for c in range(N_CORES):
        r0, r1 = c * N_LOC, (c + 1) * N_LOC
        in_maps.append({
            "encTp": encTp,
            "enclT": np.ascontiguousarray(encoding[r0:r1].T),  # [256, 1024]
            "tfm": tfm,
            "pdT": np.ascontiguousarray(
                sqr_pdist[r0:r1].T.astype(ml_dtypes.bfloat16)),  # [8192, 1024]
            "exprT": np.ascontiguousarray(
                expression[r0:r1].T.astype(ml_dtypes.bfloat16)),  # [1024, 1024]
            "g": g_bf,
        })
    return in_maps


def run(inputs, trace=False):
    if "nc" not in _cached:
        _cached["nc"] = build()
    nc = _cached["nc"]
    in_maps = _prep_inputs(**inputs)
    res = run_bass_kernel_spmd(nc, in_maps, core_ids=list(range(N_CORES)),
                               trace=trace)
    outp = np.concatenate([res.results[c]["out"] for c in range(N_CORES)],
                          axis=0)
    return outp, res


def kernel(expression, encoding, sqr_pdist, transform, gene_response):
    outp, _ = run(dict(expression=expression, encoding=encoding,
                       sqr_pdist=sqr_pdist, transform=transform,
                       gene_response=gene_response))
    return outp


# revision 17
# speedup vs baseline: 1.1037x; 1.1037x over previous
"""Trainium2 8-core kernel for nn_CellInteract.

out = ((exp(-sqr_pdist/L^2) * sigmoid(enc @ T @ enc.T)) @ expr) @ G / d_gene

Strategy:
  - Rewrite as gated @ E' with E' = expr @ G / d_gene (associativity), so the
    gated matrix feeds exactly one matmul and no transpose of the NxD partial
    product is ever needed.
  - Shard rows (cells) across 8 cores. Each core computes E' for its own row
    block (1/8 of the flops); 8 chunked AllGathers (one per 128-row tile of
    the local E') replicate it while the next tile is still being computed.
  - The main loop walks j-chunks grouped by AllGather chunk (jc = t*8 + c):
    all work gated on AllGather c happens in "c-phase" c, so compute starts
    as soon as the first chunk lands and the remaining collectives stream in
    behind the matmuls.
  - Scores are computed in transposed layout ST[j, i_local] = enc @ A.T with
    A = enc_local @ T, via float32r matmuls (TF32-like precision at full PE
    rate). That puts the contraction index j on partitions, which is the
    layout the O-matmul needs for its stationary operand.
  - Spatial gate: d in [0,1), so exp(-d/1e4) == 1 - d*1e-4 to ~5e-9; computed
    on VectorE as a fused multiply-add, keeping ScalarE free for the sigmoid.
  - O accumulates in PSUM within a c-phase and drains to an SBUF accumulator,
    freeing the PSUM banks so successive c-phases (and i-blocks) pipeline.
  - DMAs are packed 4-8 iterations per issue to keep the HWDGE sequencer off
    the critical path.
"""

import sys

for _p in ("/opt/trn_rl_repo", "/root/.axon_site"):
    if _p not in sys.path:
        sys.path.insert(0, _p)

import numpy as np
import ml_dtypes

import concourse.bacc as bacc
import concourse.mybir as mybir
import concourse.tile as tile
from concourse.bass_utils import run_bass_kernel_spmd

N = 8192
D_GENE = 1024
D_EMBED = 256
N_CORES = 8
N_LOC = N // N_CORES          # 1024 rows per core
IB = 256                      # i-block
N_IB = N_LOC // IB            # 4
JC = 128                      # j-chunk (partition dim of ST tiles)
N_JC = N // JC                # 64
NC8 = 8                       # AllGather chunks == cores
NT = N_JC // NC8              # 8 t-iterations per c-phase
TG = 4                        # t per packed DMA group
INV_SPATIAL = -1e-4           # -1/LENGTH_SCALE^2
F32 = mybir.dt.float32
F32R = mybir.dt.float32r
BF16 = mybir.dt.bfloat16

_cached = {}


def _phase_a(nc, pa, ecp, ps, rp, dp, enclT, tfm, exprT, g):
    """AT = (enc_local @ T).T in f32r; E'_local = expr_local @ G / d in bf16,
    replicated via 8 chunked AllGathers pipelined with the compute.
    Returns (at_tiles, cc_out_list)."""
    AF = mybir.ActivationFunctionType
    ALU = mybir.AluOpType

    # ---- AT[e,i] = sum_d T[d,e] * enclT[d,i]; K=D_EMBED in 2 chunks ----
    tfm_t = [pa.tile([128, D_EMBED], F32R, tag=f"tfm{k}", name=f"tfm{k}")
             for k in range(2)]
    enclT_t = [pa.tile([128, N_LOC], F32R, tag=f"enclT{k}", name=f"enclT{k}")
               for k in range(2)]
    for k in range(2):
        nc.sync.dma_start(tfm_t[k][:], tfm[k * 128:(k + 1) * 128, :])
        nc.sync.dma_start(enclT_t[k][:], enclT[k * 128:(k + 1) * 128, :])
    at = [rp.tile([128, N_LOC], F32R, tag=f"at{e}", name=f"at{e}")
          for e in range(2)]
    for e in range(2):                 # output e-chunk (partition dim)
        for ih in range(2):            # N_LOC in halves of 512
            mm = ps.tile([128, 512], F32, tag="st", name="mm")
            for k in range(2):
                nc.tensor.matmul(
                    mm[:],
                    tfm_t[k][:, e * 128:(e + 1) * 128],
                    enclT_t[k][:, ih * 512:(ih + 1) * 512],
                    start=(k == 0), stop=(k == 1),
                )
            nc.scalar.activation(
                at[e][:, ih * 512:(ih + 1) * 512], mm[:], AF.Copy)

    # ---- E'_local = expr_local @ G / d_gene, AllGathered chunk by chunk ----
    g_t = [pa.tile([128, D_GENE], BF16, tag=f"g{k}", name=f"g{k}")
           for k in range(8)]
    for k in range(8):
        nc.sync.dma_start(g_t[k][:], g[k * 128:(k + 1) * 128, :])
    exprT_r = exprT.rearrange("(k p) j -> p k j", p=128)   # [128, 8, 1024]
    cc_out = []
    for jt in range(8):
        xtp = ecp.tile([128, 8 * 128], BF16, tag="xtp", name="xtp")
        nc.sync.dma_start(
            xtp[:], exprT_r[:, :, jt * 128:(jt + 1) * 128])
        ec = ecp.tile([128, D_GENE], BF16, tag="ec", name="ec")
        for gh in range(2):
            mm = ps.tile([128, 512], F32, tag="st", name="mm")
            for k in range(8):
                nc.tensor.matmul(
                    mm[:],
                    xtp[:, k * 128:(k + 1) * 128],
                    g_t[k][:, gh * 512:(gh + 1) * 512],
                    start=(k == 0), stop=(k == 7),
                )
            nc.scalar.activation(
                ec[:, gh * 512:(gh + 1) * 512], mm[:], AF.Copy,
                scale=1.0 / D_GENE)
        cc_in_jt = dp.tile([128, D_GENE], BF16, name=f"cc_in{jt}")
        cc_out_jt = dp.tile([N_CORES * 128, D_GENE], BF16, name=f"cc_out{jt}",
                            addr_space="Shared")
        nc.scalar.dma_start(cc_in_jt[:], ec[:])
        nc.gpsimd.collective_compute(
            "AllGather",
            ALU.bypass,
            ins=[cc_in_jt.opt()],
            outs=[cc_out_jt.opt()],
            replica_groups=[list(range(N_CORES))],
        )
        cc_out.append(cc_out_jt)
    return at, cc_out


def build():
    nc = bacc.Bacc("TRN2", target_bir_lowering=False, debug=False,
                   num_devices=N_CORES)

    # encTp[p, k, j] = encoding.T[k*128+p, j]  (k-chunk packed for 1-DMA loads)
    encTp = nc.dram_tensor("encTp", [128, 2, N], F32R, kind="ExternalInput").ap()
    enclT = nc.dram_tensor("enclT", [D_EMBED, N_LOC], F32R, kind="ExternalInput").ap()
    tfm = nc.dram_tensor("tfm", [D_EMBED, D_EMBED], F32R, kind="ExternalInput").ap()
    pdT = nc.dram_tensor("pdT", [N, N_LOC], BF16, kind="ExternalInput").ap()
    exprT = nc.dram_tensor("exprT", [D_GENE, N_LOC], BF16, kind="ExternalInput").ap()
    g = nc.dram_tensor("g", [D_GENE, D_GENE], BF16, kind="ExternalInput").ap()
    out = nc.dram_tensor("out", [N_LOC, D_GENE], F32, kind="ExternalOutput").ap()

    AF = mybir.ActivationFunctionType
    ALU = mybir.AluOpType

    with tile.TileContext(nc) as tc:
        with (
            tc.tile_pool(name="res", bufs=1) as rp,
            tc.tile_pool(name="dram", bufs=1, space="DRAM") as dp,
            tc.tile_pool(name="ps", bufs=4, space="PSUM") as ps,
            tc.tile_pool(name="ops", bufs=1, space="PSUM") as ops,
        ):
            with (
                tc.tile_pool(name="pha", bufs=1) as pa,
                tc.tile_pool(name="ecp", bufs=2) as ecp,
            ):
                at, cc_out = _phase_a(nc, pa, ecp, ps, rp, dp,
                                      enclT, tfm, exprT, g)

            # O accumulators in SBUF (f32), one per i-block
            osb = [rp.tile([128, D_GENE], F32, tag=f"osb{si}",
                           name=f"osb{si}") for si in range(2 * N_IB)]

            # pdT rows j = t*1024 + c*128 + p  ->  [p, t, c, i]
            pdT_r = pdT.rearrange("(t c p) i -> p t c i", p=128, c=NC8)
            # encTp cols j = t*1024 + c*128 + jj  ->  [p, k, t, c, jj]
            encT_r = encTp.rearrange("p k (t c jj) -> p k t c jj",
                                     t=NT, c=NC8)

            # ---------------- main loop ----------------
            # i handled in pairs of blocks (IBP=512) so score matmuls run at
            # N=512; O accumulation split by g-half so PSUM stays at 8 banks
            # (4 x o + 4 x st). Gating tiles are computed once per (c, ibp)
            # and reused by both g-half passes. The scores/gating for the
            # first PRE c-phases are emitted before any O-matmul, giving the
            # PE useful work while the first AllGathers are still in flight.
            with tc.tile_pool(name="str", bufs=1) as ms:
                def gating(c, ibp):
                    """score + sigmoid + spatial gate for one (c, ibp);
                    returns the 8 gated lhsT tiles."""
                    i0 = ibp * 512
                    ekc = ms.tile([128, 2 * NT * JC], F32R,
                                  tag="ekc", name=f"ekc{c}", bufs=2)
                    nc.sync.dma_start(ekc[:], encT_r[:, :, :, c, :])
                    gts = []
                    for tg in range(2):
                        t0 = tg * TG
                        pdp = ms.tile([128, TG * 512], BF16,
                                      tag="pdp", name="pdp", bufs=2)
                        nc.sync.dma_start(
                            pdp[:], pdT_r[:, t0:t0 + TG, c, i0:i0 + 512])
                        spatp = ms.tile([128, TG * 512], BF16,
                                        tag="spatp", name="spatp", bufs=2)
                        nc.vector.tensor_scalar(
                            spatp[:], pdp[:], INV_SPATIAL, 1.0,
                            ALU.mult, ALU.add)
                        for dt in range(TG):
                            t = t0 + dt
                            st = ps.tile([JC, 512], F32, tag="st", name="st")
                            for k in range(2):
                                nc.tensor.matmul(
                                    st[:],
                                    ekc[:, (k * NT + t) * JC:
                                        (k * NT + t + 1) * JC],
                                    at[k][:, i0:i0 + 512],
                                    start=(k == 0), stop=(k == 1))
                            sig = ms.tile([JC, 512], BF16,
                                          tag="sig", name="sig", bufs=8)
                            nc.scalar.activation(sig[:], st[:], AF.Sigmoid)
                            gt = ms.tile([JC, 512], BF16,
                                         tag="gt", name="gt", bufs=17)
                            nc.vector.tensor_mul(
                                gt[:], sig[:], spatp[:, dt * 512:(dt + 1) * 512])
                            gts.append(gt)
                    return gts

                def load_ep(c):
                    # E' slice for AG chunk c: epc[:, t*1024:] = rank t's rows
                    # of chunked AllGather c (= E'[t*1024 + c*128 ..]). Loaded
                    # via GpSimd (SWDGE) so the collective-completion wait
                    # does not head-of-line-block the sync HWDGE queue.
                    epc = ms.tile([128, NT * D_GENE], BF16,
                                  tag="ep", name=f"ep{c}", bufs=2)
                    for t in range(NT):
                        nc.gpsimd.dma_start(
                            epc[:, t * D_GENE:(t + 1) * D_GENE],
                            cc_out[c][t * 128:(t + 1) * 128, :])
                    return epc

                def o_phase(c, ibp, epc, gts):
                    for gh in range(2):
                        o_ps = [ops.tile([128, 512], F32, tag=f"o{si}",
                                         name=f"o{si}") for si in range(4)]
                        for t in range(NT):
                            for si in range(4):
                                nc.tensor.matmul(
                                    o_ps[si][:],
                                    gts[t][:, si * 128:(si + 1) * 128],
                                    epc[:, t * D_GENE + gh * 512:
                                         t * D_GENE + (gh + 1) * 512],
                                    start=(t == 0), stop=(t == NT - 1),
                                )
                        for si in range(4):
                            dst = osb[4 * ibp + si][:, gh * 512:(gh + 1) * 512]
                            if c == 0:
                                nc.vector.tensor_copy(dst, o_ps[si][:])
                            else:
                                nc.vector.tensor_add(dst, dst, o_ps[si][:])

                for c in range(NC8):
                    epc = load_ep(c)
                    for ibp in range(2):
                        o_phase(c, ibp, epc, gating(c, ibp))

                # write out
                for si in range(2 * N_IB):
                    nc.sync.dma_start(
                        out[si * 128:(si + 1) * 128, :], osb[si][:])

    nc.compile()
    return nc


def _prep_inputs(expression, encoding, sqr_pdist, transform, gene_response):
    expression = np.asarray(expression, dtype=np.float32)
    encoding = np.asarray(encoding, dtype=np.float32)
    sqr_pdist = np.asarray(sqr_pdist, dtype=np.float32)
    transform = np.asarray(transform, dtype=np.float32)
    gene_response = np.asarray(gene_response, dtype=np.float32)

    encT = np.ascontiguousarray(encoding.T)                    # [256, 8192]
    encTp = np.ascontiguousarray(encT.reshape(2, 128, N).transpose(1, 0, 2))
    tfm = np.ascontiguousarray(transform)                      # [256, 256]
    g_bf = np.ascontiguousarray(gene_response.astype(ml_dtypes.bfloat16))
    in_maps = []
    for c in range(N_CORES):
        r0, r1 = c * N_LOC, (c + 1) * N_LOC
        in_maps.append({
            "encTp": encTp,
            "enclT": np.ascontiguousarray(encoding[r0:r1].T),  # [256, 1024]
            "tfm": tfm,
            "pdT": np.ascontiguousarray(
                sqr_pdist[r0:r1].T.astype(ml_dtypes.bfloat16)),  # [8192, 1024]
            "exprT": np.ascontiguousarray(
                expression[r0:r1].T.astype(ml_dtypes.bfloat16)),  # [1024, 1024]
            "g": g_bf,
        })
    return in_maps


def run(inputs, trace=False):
    if "nc" not in _cached:
        _cached["nc"] = build()
    nc = _cached["nc"]
    in_maps = _prep_inputs(**inputs)
    res = run_bass_kernel_spmd(nc, in_maps, core_ids=list(range(N_CORES)),
                               trace=trace)
    outp = np.concatenate([res.results[c]["out"] for c in range(N_CORES)],
                          axis=0)
    return outp, res


def kernel(expression, encoding, sqr_pdist, transform, gene_response):
    outp, _ = run(dict(expression=expression, encoding=encoding,
                       sqr_pdist=sqr_pdist, transform=transform,
                       gene_response=gene_response))
    return outp
